# revision 1
# baseline (speedup 1.0000x reference)
"""Trainium2 Bass kernel for DCN_ConvLSTM2D.

Math (per batch element, data-parallel over 8 cores):
  om    = conv3x3(x, w_off) + b_off            -> dy, dx, mask=sigmoid
  x_cat = modulated deformable conv (DCNv2): bilinear-sample x at
          (h+kh+dy, w+kw+dx) per kernel point, scale by mask, then
          contract with w_dcn over (cin, k) and add b_dcn
  h_cat = conv3x3(h, w_h)
  LSTM gates with peephole mul_c; outputs (h_next, c_next).

Implementation notes:
  * Bilinear sampling is rewritten as an exact 5x5 "tent window":
      sample_k[ch,p] = sum_{u,v in -2..2} tY_u(dy)[p] tX_v(dx)[p]
                       * xpad[ch, p + (kh+u, kw+v)]
    where the tent values for |off| <= 2 are exactly
      tY_{-2}=b2, tY_{-1}=b1-2*b2, tY_0=relu(1-a1-b1),
      tY_{1}=a1-2*a2, tY_{2}=a2
    with a1=relu(dy), a2=relu(dy-1), b1=relu(-dy), b2=relu(-dy-1).
    Zero padding of xpad reproduces the reference's out-of-bounds
    zeroing exactly.
  * The 225 per-pixel coefficient maps (mask * tY_u * tX_v per k) are
    built on-chip, staged to a DRAM scratch, and row-broadcast back to
    [64, 4096] tiles (partition-broadcast DMA is only legal from DRAM).
  * x_cat + h_cat are fused in PSUM accumulation (the LSTM adds them),
    and b_dcn is folded into the gate activations' bias.
"""

import numpy as np

import concourse.bacc as bacc
import concourse.mybir as mybir
import concourse.tile as tile
from concourse.bass_utils import run_bass_kernel_spmd

F32 = mybir.dt.float32
F16 = mybir.dt.float16
AF = mybir.ActivationFunctionType
OP = mybir.AluOpType

B, C, H, W = 8, 64, 64, 64
HW = H * W
KK = 9
XP = 70   # x padded to [70, 70] (pad 3: kernel offset 1 + window 2)
HP = 66   # h padded to [66, 66] (pad 1)
NBLK = 8  # pixel blocks of 512 (8 image rows)
BLK = 512

_COMPILED = {}


def _build(terms=()):
    nc = bacc.Bacc(None, target_bir_lowering=False)

    x_in = nc.dram_tensor("x", [C, HW], F32, kind="ExternalInput")
    h_in = nc.dram_tensor("h", [C, HW], F32, kind="ExternalInput")
    c_in = nc.dram_tensor("c", [C, HW], F32, kind="ExternalInput")
    mulc_if_in = nc.dram_tensor("mulc_if", [128, HW], F32, kind="ExternalInput")
    mulc_o_in = nc.dram_tensor("mulc_o", [64, HW], F32, kind="ExternalInput")
    woff_in = nc.dram_tensor("woff", [64, KK, 27], F32, kind="ExternalInput")
    boff_in = nc.dram_tensor("boff", [27, 1], F32, kind="ExternalInput")
    wdcn_in = nc.dram_tensor("wdcn", [128, 5, 256], F16, kind="ExternalInput")
    bdcn_in = nc.dram_tensor("bdcn", [128, 2], F32, kind="ExternalInput")
    wh_in = nc.dram_tensor("wh", [64, KK, 256], F32, kind="ExternalInput")

    psi_dram = nc.dram_tensor("psi_scratch", [81, HW], F16)
    NT = max(1, len(terms))
    corr_dram = nc.dram_tensor("corr_scratch", [1, NT], F32)
    om_dram = nc.dram_tensor("om_scratch", [27, HW], F32)

    h_out = nc.dram_tensor("h_out", [C, HW], F32, kind="ExternalOutput")
    c_out = nc.dram_tensor("c_out", [C, HW], F32, kind="ExternalOutput")

    with tile.TileContext(nc) as tc:
        with tc.tile_pool(name="persist", bufs=1) as pp:
            xpad = pp.tile([C, XP * XP], F32, tag="xpad")
            hpad = pp.tile([C, HP * HP], F32, tag="hpad")
            c2 = pp.tile([128, HW], F32, tag="c2")
            S = pp.tile([128, 4, HW], F16, tag="S")
            S4 = pp.tile([64, HW], F16, tag="S4")
            xp16 = pp.tile([C, XP * XP], F16, tag="xp16")
            xp16b = pp.tile([C, XP * XP], F16, tag="xp16b")
            woff = pp.tile([64, KK, 27], F32, tag="woff")
            consts = pp.tile([128, 4], F32, tag="consts")
            corr_bc = pp.tile([64, NT], F32, tag="corr_bc")
            boff = consts[0:27, 0:1]
            bdcn0 = consts[:, 1:2]
            nc.vector.memset(consts[:, 3:4], -1.0)

            nc.vector.memset(xpad[:], 0.0)
            nc.vector.memset(hpad[:], 0.0)

            xpv = xpad[:].rearrange("p (r c) -> p r c", c=XP)
            hpv = hpad[:].rearrange("p (r c) -> p r c", c=HP)
            nc.sync.dma_start(xpv[:, 3 : 3 + H, 3 : 3 + W],
                              x_in[:].rearrange("p (r c) -> p r c", c=W))
            nc.sync.dma_start(hpv[:, 1 : 1 + H, 1 : 1 + W],
                              h_in[:].rearrange("p (r c) -> p r c", c=W))
            nc.sync.dma_start(c2[0:64, :], c_in[:])
            nc.sync.dma_start(c2[64:128, :], c_in[:])
            nc.sync.dma_start(woff[:], woff_in[:])
            nc.sync.dma_start(boff, boff_in[:])
            nc.sync.dma_start(consts[:, 1:3], bdcn_in[:])

            nc.vector.tensor_copy(xp16[:], xpad[:])
            nc.vector.tensor_copy(xp16b[:, 0 : XP * XP - 1], xpad[:, 1 : XP * XP])
            xv16 = xp16[:].rearrange("p (r c) -> p r c", c=XP)
            xv16b = xp16b[:].rearrange("p (r c) -> p r c", c=XP)

            # ---- Phase 1: offset conv + coefficient maps -> psi_dram ----
            # Map workspace layout: [36, 1024] tiles, row = k*4 + q where
            # pixel p = q*1024 + col (q = quarter of the image).
            with tc.tile_pool(name="maps", bufs=1) as mp:
                dy36 = mp.tile([36, 1024], F32, tag="dy36")
                dx36 = mp.tile([36, 1024], F32, tag="dx36")
                msk = mp.tile([36, 1024], F32, tag="msk")
                tY = [mp.tile([36, 1024], F32, tag=f"tY{u}", name=f"tY{u}") for u in range(5)]
                tX = [mp.tile([36, 1024], F32, tag=f"tX{u}", name=f"tX{u}") for u in range(5)]

                with (
                    tc.tile_pool(name="omp", bufs=2) as omp_,
                    tc.tile_pool(name="psum_om", bufs=2, space="PSUM") as psom,
                ):
                    for q in range(4):
                        omq = omp_.tile([27, 1024], F32, tag="om_q")
                        for hb in range(2):
                            blk = q * 2 + hb
                            ps = psom.tile([27, BLK], F32, tag="omps")
                            for t in range(KK):
                                ky, kx = t // 3, t % 3
                                # x row r + ky - 1 -> xpad row r + ky + 2
                                rhs = xpv[:, blk * 8 + ky + 2 : blk * 8 + ky + 10,
                                          kx + 2 : kx + 2 + W]
                                nc.tensor.matmul(ps[:], woff[:, t, :], rhs,
                                                 start=(t == 0),
                                                 stop=(t == KK - 1))
                            nc.scalar.activation(
                                omq[:, hb * BLK : (hb + 1) * BLK], ps[:],
                                AF.Identity, bias=boff, scale=1.0)
                        nc.sync.dma_start(
                            om_dram[:, q * 1024 : (q + 1) * 1024], omq[:])
                    # repack via DRAM: [9, (4, 1024)] -> [36, 1024]
                    for (dst, r0) in ((dy36, 0), (dx36, 9), (msk, 18)):
                        nc.sync.dma_start(
                            dst[:],
                            om_dram[r0 : r0 + 9, :].rearrange(
                                "p (q f) -> (p q) f", q=4))

                nc.scalar.activation(msk[:], msk[:], AF.Sigmoid)
                for (src, tT) in ((dy36, tY), (dx36, tX)):
                    # tT[j] = tent value at u = j - 2; built in place:
                    nc.scalar.activation(tT[3][:], src[:], AF.Relu)       # a1
                    nc.scalar.activation(tT[1][:], src[:], AF.Relu,
                                         scale=-1.0)                      # b1
                    nc.scalar.activation(tT[4][:], src[:], AF.Relu,
                                         bias=consts[0:36, 3:4])              # a2
                    nc.scalar.activation(tT[0][:], src[:], AF.Relu, scale=-1.0,
                                         bias=consts[0:36, 3:4])              # b2
                    nc.vector.tensor_add(tT[2][:], tT[3][:], tT[1][:])    # a1+b1
                    nc.scalar.activation(tT[2][:], tT[2][:], AF.Relu,
                                         scale=-1.0, bias=1.0)  # relu(1-a1-b1)
                    nc.vector.scalar_tensor_tensor(tT[3][:], tT[4][:], -2.0,
                                                   tT[3][:], OP.mult, OP.add)
                    nc.vector.scalar_tensor_tensor(tT[1][:], tT[0][:], -2.0,
                                                   tT[1][:], OP.mult, OP.add)

                for ub in range(5):  # fold mask into the Y-side factors
                    nc.vector.tensor_mul(tY[ub][:], tY[ub][:], msk[:])

                psi16 = mp.tile([36, 1024], F16, tag="psi16")
                for ub in range(1, 4):
                    for vb in range(1, 4):
                        nc.vector.tensor_mul(psi16[:], tY[ub][:], tX[vb][:])
                        row = ((ub - 1) * 3 + (vb - 1)) * 9
                        nc.sync.dma_start(
                            psi_dram[row : row + 9, :].rearrange(
                                "p (q f) -> (p q) f", q=4),
                            psi16[:])

                # sparse tail corrections (window positions at u or v = +-2):
                # gather the per-violator coefficient factors, multiply, and
                # broadcast to all 64 partitions via DRAM. Terms whose pixels
                # don't actually violate on this core have a zero factor, so
                # the later applies are exact no-ops (SPMD-safe union).
                if terms:
                    stgA = mp.tile([1, NT], F32, tag="stgA")
                    stgB = mp.tile([1, NT], F32, tag="stgB")
                    for i, (k, ja, jb, mrow, mcol, xflat, p) in enumerate(terms):
                        nc.sync.dma_start(
                            stgA[:, i : i + 1],
                            tY[ja][mrow : mrow + 1, mcol : mcol + 1])
                        nc.sync.dma_start(
                            stgB[:, i : i + 1],
                            tX[jb][mrow : mrow + 1, mcol : mcol + 1])
                    nc.vector.tensor_mul(stgA[:], stgA[:], stgB[:])
                    nc.sync.dma_start(corr_dram[:], stgA[:])
                    nc.sync.dma_start(
                        corr_bc[:],
                        corr_dram[0:1, :].to_broadcast([64, NT]))

            # ---- Phase 2: sampling MACs into S ----
            # Accumulate each k at base partition 0 (HW requires equal base
            # partitions for two-SBUF-input vector ops), then DMA odd k's
            # into the upper half of its S chunk.
            with tc.tile_pool(name="macp", bufs=2) as macp_:
                with tc.tile_pool(name="bc", bufs=4) as bcp:
                    t_tile = macp_.tile([64, HW], F16, tag="t", bufs=2)
                    for k in range(KK):
                        kh, kw = k // 3 - 1, k % 3 - 1
                        q, half = k // 2, k % 2
                        if k == 8:
                            s64 = S4[:]
                        elif half == 0:
                            s64 = S[0:64, q, :]
                        else:
                            s64 = macp_.tile([64, HW], F16, tag="s64",
                                             name="s64", bufs=1)
                        first = True
                        for u in (-1, 0, 1):
                            for v in (-1, 0, 1):
                                row = ((u + 1) * 3 + (v + 1)) * 9 + k
                                bc = bcp.tile([64, HW], F16, tag="bc")
                                nc.sync.dma_start(
                                    bc[:],
                                    psi_dram[row : row + 1, :].to_broadcast([64, HW]))
                                # pick the padded copy that keeps the fp16
                                # read 4B-aligned (DVE 2x mode requirement)
                                r0, c0 = 3 + kh + u, 3 + kw + v
                                if c0 % 2 == 0:
                                    xsh = xv16[:, r0 : r0 + H, c0 : c0 + W]
                                else:
                                    xsh = xv16b[:, r0 : r0 + H,
                                                c0 - 1 : c0 - 1 + W]
                                if first:
                                    nc.vector.tensor_mul(s64, bc[:], xsh)
                                    first = False
                                else:
                                    nc.vector.tensor_mul(t_tile[:], bc[:], xsh)
                                    nc.vector.tensor_add(s64, s64, t_tile[:])
                        for i, (tk, ja, jb, mrow, mcol, xflat, p) in \
                                enumerate(terms):
                            if tk != k:
                                continue
                            nc.vector.scalar_tensor_tensor(
                                s64[:, p : p + 1],
                                xp16[:, xflat : xflat + 1],
                                corr_bc[:, i : i + 1],
                                s64[:, p : p + 1],
                                OP.mult, OP.add)
                        if half == 1:
                            nc.sync.dma_start(S[64:128, q, :], s64)

            # ---- Phase 3: fused DCN + h-conv matmul, gates, outputs ----
            with (
                tc.tile_pool(name="gates", bufs=1) as gp,
                tc.tile_pool(name="gwork", bufs=1) as gw,
                tc.tile_pool(name="psum_g", bufs=4, space="PSUM") as psg,
            ):
                mulc_if = gp.tile([128, HW], F32, tag="mulc_if")
                mulc_o = gp.tile([64, HW], F32, tag="mulc_o")
                wdcn = gp.tile([128, 5, 256], F16, tag="wdcn")
                wh = gp.tile([64, KK, 256], F32, tag="wh")
                nc.sync.dma_start(mulc_if[:], mulc_if_in[:])
                nc.sync.dma_start(mulc_o[:], mulc_o_in[:])
                nc.sync.dma_start(wdcn[:], wdcn_in[:])
                nc.sync.dma_start(wh[:], wh_in[:])

                for blk in range(NBLK):
                    lo, hi = blk * BLK, (blk + 1) * BLK
                    ps0 = psg.tile([128, BLK], F32, tag="ps0")
                    ps1 = psg.tile([128, BLK], F32, tag="ps1")
                    for half, ps in ((0, ps0), (1, ps1)):
                        hs = half * 128
                        for q in range(4):
                            nc.tensor.matmul(ps[:], wdcn[:, q, hs : hs + 128],
                                             S[:, q, lo:hi],
                                             start=(q == 0), stop=False)
                        nc.tensor.matmul(ps[:], wdcn[0:64, 4, hs : hs + 128],
                                         S4[:, lo:hi], start=False, stop=False)
                        for t in range(KK):
                            ky, kx = t // 3, t % 3
                            rhs = hpv[:, blk * 8 + ky : blk * 8 + ky + 8,
                                      kx : kx + W]
                            nc.tensor.matmul(ps[:], wh[:, t, hs : hs + 128], rhs,
                                             start=False, stop=(t == KK - 1))

                    tif = gw.tile([128, BLK], F32, tag="tif")
                    uif = gw.tile([128, BLK], F32, tag="uif")
                    ift = gw.tile([128, BLK], F32, tag="ift")
                    cgc = gw.tile([128, BLK], F32, tag="cgc")
                    prod = gw.tile([128, BLK], F32, tag="prod")
                    pf = gw.tile([64, BLK], F32, tag="pf")
                    cnx = gw.tile([64, BLK], F32, tag="cnx")
                    to_ = gw.tile([64, BLK], F32, tag="to")
                    uo = gw.tile([64, BLK], F32, tag="uo")
                    ot = gw.tile([64, BLK], F32, tag="ot")
                    rc = gw.tile([64, BLK], F32, tag="rc")
                    hnx = gw.tile([64, BLK], F32, tag="hnx")

                    nc.vector.tensor_mul(tif[:], mulc_if[:, lo:hi], c2[:, lo:hi])
                    nc.vector.scalar_tensor_tensor(uif[:], ps0[:], 1.0, tif[:],
                                                   OP.mult, OP.add)
                    nc.scalar.activation(ift[:], uif[:], AF.Sigmoid,
                                         bias=bdcn0)
                    nc.scalar.activation(cgc[0:64, :], ps1[0:64, :], AF.Relu,
                                         bias=consts[0:64, 2:3])
                    nc.scalar.activation(cgc[64:128, :], c2[64:128, lo:hi],
                                         AF.Copy)
                    # split the i*c_gate / f*c products so the final add has
                    # equal input base partitions (HW constraint)
                    nc.vector.tensor_mul(prod[0:64, :], ift[0:64, :],
                                         cgc[0:64, :])
                    nc.vector.tensor_mul(pf[:], ift[64:128, :], cgc[64:128, :])
                    nc.vector.tensor_add(cnx[:], prod[0:64, :], pf[:])
                    nc.vector.tensor_mul(to_[:], mulc_o[:, lo:hi], cnx[:])
                    nc.vector.scalar_tensor_tensor(uo[:], ps1[64:128, :], 1.0,
                                                   to_[:], OP.mult, OP.add)
                    nc.scalar.activation(ot[:], uo[:], AF.Sigmoid,
                                         bias=consts[64:128, 2:3])
                    nc.scalar.activation(rc[:], cnx[:], AF.Relu)
                    nc.vector.tensor_mul(hnx[:], ot[:], rc[:])
                    nc.sync.dma_start(c_out[:, lo:hi], cnx[:])
                    nc.sync.dma_start(h_out[:, lo:hi], hnx[:])

    nc.compile()
    return nc


def compute_terms(x, w_off, b_off, thresh=0.95):
    """Violator index structure from a host-side replica of the offset conv.

    Only *indices* are host-derived (with a safe threshold margin); every
    numeric value the corrections use is computed on device, so terms that
    don't violate on-device contribute exactly zero.
    """
    x = np.asarray(x, np.float32)
    w = np.asarray(w_off, np.float32)
    bb = np.asarray(b_off, np.float32)
    xp = np.pad(x, ((0, 0), (0, 0), (1, 1), (1, 1)))
    om = np.zeros((B, 3 * KK, H, W), np.float32)
    for ky in range(3):
        for kx in range(3):
            om += np.einsum("oc,bchw->bohw", w[:, :, ky, kx],
                            xp[:, :, ky : ky + H, kx : kx + W],
                            optimize=True)
    om += bb[None, :, None, None]
    dy, dx = om[:, :KK], om[:, KK : 2 * KK]

    # positions (k, p, ue, ve) of missing 5x5-window taps, as a set
    pos = set()
    for arr, is_y in ((dy, True), (dx, False)):
        bidx, kidx, ridx, widx = np.nonzero(np.abs(arr) > thresh)
        for b, k, r, w_ in zip(bidx, kidx, ridx, widx):
            sgn = 1 if arr[b, k, r, w_] > 0 else -1
            p = int(r) * W + int(w_)
            if is_y:
                for ve in (-1, 0, 1):
                    pos.add((int(k), p, 2 * sgn, ve))
            else:
                for ue in (-1, 0, 1):
                    pos.add((int(k), p, ue, 2 * sgn))
    # corner positions where both axes may violate
    ys = {}
    xs = {}
    for arr, d in ((dy, ys), (dx, xs)):
        bidx, kidx, ridx, widx = np.nonzero(np.abs(arr) > thresh)
        for b, k, r, w_ in zip(bidx, kidx, ridx, widx):
            key = (int(k), int(r) * W + int(w_))
            d.setdefault(key, set()).add(1 if arr[b, k, r, w_] > 0 else -1)
    for key in set(ys) & set(xs):
        for sy in ys[key]:
            for sx in xs[key]:
                pos.add((key[0], key[1], 2 * sy, 2 * sx))

    terms = []
    for (k, p, ue, ve) in sorted(pos):
        kh, kw = k // 3 - 1, k % 3 - 1
        ja = ue + 2
        jb = ve + 2
        mrow = k * 4 + p // 1024
        mcol = p % 1024
        r, w_ = p // W, p % W
        xflat = (3 + kh + ue + r) * XP + (3 + kw + ve + w_)
        terms.append((k, ja, jb, mrow, mcol, xflat, p))
    return tuple(terms)


def get_nc(terms=()):
    if terms not in _COMPILED:
        _COMPILED[terms] = _build(terms)
    return _COMPILED[terms]


def make_in_maps(x, h, c, w_off, b_off, w_dcn, b_dcn, w_h, mul_c):
    x = np.ascontiguousarray(x, np.float32)
    h = np.ascontiguousarray(h, np.float32)
    c = np.ascontiguousarray(c, np.float32)
    mul_c = np.asarray(mul_c, np.float32)

    mulc_if = np.ascontiguousarray(mul_c[0, 0:128].reshape(128, HW))
    mulc_o = np.ascontiguousarray(mul_c[0, 128:192].reshape(64, HW))
    woff = np.ascontiguousarray(
        np.asarray(w_off, np.float32).reshape(27, 64, KK).transpose(1, 2, 0))
    boff = np.asarray(b_off, np.float32).reshape(27, 1)
    # wdcn rows ordered (k*64+ch); padded to 640 rows -> [5][128, 256] chunks
    wd = np.asarray(w_dcn, np.float32).reshape(256, 64, KK).transpose(2, 1, 0)
    wd = wd.reshape(576, 256)
    wdp = np.zeros((640, 256), np.float32)
    wdp[:576] = wd
    wdcn = np.ascontiguousarray(
        wdp.reshape(5, 128, 256).transpose(1, 0, 2)).astype(np.float16)
    bdcn = np.ascontiguousarray(
        np.asarray(b_dcn, np.float32).reshape(2, 128).T)  # [128, 2]
    whp = np.ascontiguousarray(
        np.asarray(w_h, np.float32).reshape(256, 64, KK).transpose(1, 2, 0))

    shared = dict(mulc_if=mulc_if, mulc_o=mulc_o, woff=woff, boff=boff,
                  wdcn=wdcn, bdcn=bdcn, wh=whp)
    in_maps = []
    for b in range(B):
        m = dict(shared)
        m["x"] = x[b].reshape(C, HW)
        m["h"] = h[b].reshape(C, HW)
        m["c"] = c[b].reshape(C, HW)
        in_maps.append(m)
    return in_maps


def kernel(x, h, c, w_off, b_off, w_dcn, b_dcn, w_h, mul_c):
    terms = compute_terms(x, w_off, b_off)
    nc = get_nc(terms)
    in_maps = make_in_maps(x, h, c, w_off, b_off, w_dcn, b_dcn, w_h, mul_c)
    res = run_bass_kernel_spmd(nc, in_maps, core_ids=list(range(B)))
    h_next = np.stack([res.results[b]["h_out"].reshape(C, H, W)
                       for b in range(B)])
    c_next = np.stack([res.results[b]["c_out"].reshape(C, H, W)
                       for b in range(B)])
    return h_next.astype(np.float32), c_next.astype(np.float32)



# revision 7
# speedup vs baseline: 2.7158x; 2.7158x over previous
"""Trainium2 Bass kernel for DCN_ConvLSTM2D (v2 — fused matmul pipeline).

Math (per batch element, data-parallel over 8 cores):
  om    = conv3x3(x, w_off) + b_off            -> dy, dx, mask=sigmoid
  x_cat = modulated deformable conv (DCNv2)
  h_cat = conv3x3(h, w_h)
  LSTM gates with peephole mul_c; outputs (h_next, c_next).

v2 design (vs the S-materializing baseline):
  * Bilinear sampling via the exact tent window. Dense taps
    (u,v) in {-1,0,1}^2: DVE products R = psi_bc * x_window stream
    directly into the DCN matmul accumulation (PSUM sums the taps) —
    no S tensor and no DVE adds.
  * Tap pairing: x lives in [128, 70*70] tiles whose upper 64
    partitions hold the image shifted by +1 col (xpC) or +1 row (xpR),
    plus 1-element-shifted b-variants keeping fp16 DVE reads 4B
    aligned. One [128,*] DVE product covers TWO taps; the stationary
    (w_k; w_k) contracts both halves: 5 operands/k instead of 9.
  * All matmul paths fp16 (om conv, h conv, DCN).
  * |offset|>1 tail taps: host evaluates their tent coefficients
    (indices AND values; ~1e-3 off device numerics, far inside the
    2e-2 gate). Device gathers x-neighbor columns with indirect DMA,
    contracts them against w_dcn in ~10 small matmuls, scatters
    per-pixel patch rows to DRAM with 2 indirect DMAs, and
    transpose-loads patch maps that the gate stage adds in 3 wide DVE
    ops per block. Other cores' terms carry zero coefficients
    (SPMD-safe union).
"""

import numpy as np

import concourse.bacc as bacc
import concourse.bass as bass
import concourse.mybir as mybir
import concourse.tile as tile
from concourse.bass_utils import run_bass_kernel_spmd
from concourse.masks import make_identity

F32 = mybir.dt.float32
F16 = mybir.dt.float16
I32 = mybir.dt.int32
AF = mybir.ActivationFunctionType
OP = mybir.AluOpType

B, C, H, W = 8, 64, 64, 64
HW = H * W
KK = 9
XP = 70    # x padded grid (pad 3)
HP = 66    # h padded grid (pad 1)
NUP = 256  # padded correction-column count
NC4 = 4 * NUP
PDC = 320  # patch DRAM row width: 128 (if) + 64 (c) + 128 (o at cols 192:256 + pad)

_COMPILED = {}


def _row(k, u, v):
    return ((u + 1) * 3 + (v + 1)) * KK + k


def _k_ops(k):
    """Dense-tap operand table: (pair?, xsel, r0, c0, row_lo, row_hi)."""
    kh, kw = k // 3 - 1, k % 3 - 1
    ops = []
    for u in (-1, 0, 1):  # v-pair ((u,-1) lower, (u,0) upper) via xpC
        ops.append((True, "C", 3 + kh + u, 2 + kw,
                    _row(k, u, -1), _row(k, u, 0)))
    # u-pair ((-1,+1) lower, (0,+1) upper) via xpR
    ops.append((True, "R", 2 + kh, 4 + kw, _row(k, -1, 1), _row(k, 0, 1)))
    # solo (+1,+1): lower 64 partitions, upper zeroed
    ops.append((False, "C", 4 + kh, 4 + kw, _row(k, 1, 1), None))
    return ops


def _build(granges, ov):
    nc = bacc.Bacc(None, target_bir_lowering=False)

    x16_in = nc.dram_tensor("x16", [C, HW], F16, kind="ExternalInput")
    xt16_in = nc.dram_tensor("xt16", [HW, C], F16, kind="ExternalInput")
    h16_in = nc.dram_tensor("h16", [C, HW], F16, kind="ExternalInput")
    c_in = nc.dram_tensor("cf", [C, HW], F32, kind="ExternalInput")
    mulc_if_in = nc.dram_tensor("mulc_if", [128, HW], F16, kind="ExternalInput")
    mulc_o_in = nc.dram_tensor("mulc_o", [64, HW], F16, kind="ExternalInput")
    woff_in = nc.dram_tensor("woff", [64, KK, 27], F16, kind="ExternalInput")
    boff_in = nc.dram_tensor("boff", [27, 1], F32, kind="ExternalInput")
    wdcn_in = nc.dram_tensor("wdcnp", [128, KK, 256], F16, kind="ExternalInput")
    wu_in = nc.dram_tensor("wu", [128, 5, 256], F16, kind="ExternalInput")
    bdcn_in = nc.dram_tensor("bdcn", [128, 2], F32, kind="ExternalInput")
    whp_in = nc.dram_tensor("whp", [128, 5, 256], F16, kind="ExternalInput")
    corr_in = nc.dram_tensor("corr4", [2, NC4], F16, kind="ExternalInput")
    gidx_in = nc.dram_tensor("gidx", [128, NC4 // 128], I32, kind="ExternalInput")
    sidx_in = nc.dram_tensor("sidx", [128, NUP // 128], I32, kind="ExternalInput")

    om_dram = nc.dram_tensor("om_scratch", [27, HW], F32)
    psi_dram = nc.dram_tensor("psi_scratch", [81, HW], F16)
    pd = nc.dram_tensor("patch_scratch", [HW, PDC], F16)

    h_out = nc.dram_tensor("h_out", [C, HW], F32, kind="ExternalOutput")
    c_out = nc.dram_tensor("c_out", [C, HW], F32, kind="ExternalOutput")

    n_terms = sum(e - s for s, e in granges)

    with tile.TileContext(nc) as tc:
        with tc.tile_pool(name="persist", bufs=1) as pp:
            xpC = pp.tile([128, XP * XP], F16, tag="xpC")
            xpCb = pp.tile([128, XP * XP], F16, tag="xpCb")
            xpR = pp.tile([128, XP * XP], F16, tag="xpR")
            xpRb = pp.tile([128, XP * XP], F16, tag="xpRb")
            hpC = pp.tile([128, HP * HP], F16, tag="hpC")
            hpR = pp.tile([128, HP * HP], F16, tag="hpR")
            c2 = pp.tile([128, HW], F32, tag="c2")
            mulc_if = pp.tile([128, HW], F16, tag="mulc_if")
            mulc_o = pp.tile([64, HW], F16, tag="mulc_o")
            woff = pp.tile([64, KK, 27], F16, tag="woff")
            wdcn = pp.tile([128, KK, 256], F16, tag="wdcn")
            whp = pp.tile([128, 5, 256], F16, tag="whp")
            consts = pp.tile([128, 4], F32, tag="consts")
            PALL0 = pp.tile([128, HW], F16, tag="PALL0")   # if-patch
            PCOc = pp.tile([128, HW], F16, tag="PCOc")     # rows 0:64 = c-patch
            PCOo = pp.tile([128, HW], F16, tag="PCOo")     # rows 0:64 = o-patch
            boff = consts[0:27, 0:1]
            bdcn0 = consts[:, 1:2]

            for t in (xpC, xpCb, xpR, xpRb, hpC, hpR, PALL0, PCOc, PCOo):
                nc.vector.memset(t[:], 0.0)
            nc.vector.memset(consts[:, 3:4], -1.0)

            xg = {n: t[:].rearrange("p (r c) -> p r c", c=XP)
                  for n, t in (("C", xpC), ("Cb", xpCb),
                               ("R", xpR), ("Rb", xpRb))}
            hgC = hpC[:].rearrange("p (r c) -> p r c", c=HP)
            hgR = hpR[:].rearrange("p (r c) -> p r c", c=HP)
            xin = x16_in[:].rearrange("p (r c) -> p r c", c=W)
            hin = h16_in[:].rearrange("p (r c) -> p r c", c=W)

            for gview, (lo, uo) in ((xg["C"], ((3, 3), (3, 2))),
                                    (xg["Cb"], ((3, 2), (3, 1))),
                                    (xg["R"], ((3, 3), (2, 3))),
                                    (xg["Rb"], ((3, 2), (2, 2)))):
                nc.sync.dma_start(gview[0:64, lo[0]:lo[0] + H,
                                        lo[1]:lo[1] + W], xin)
                nc.sync.dma_start(gview[64:128, uo[0]:uo[0] + H,
                                        uo[1]:uo[1] + W], xin)
            nc.sync.dma_start(hgC[0:64, 1:1 + H, 1:1 + W], hin)
            nc.sync.dma_start(hgC[64:128, 1:1 + H, 0:0 + W], hin)
            nc.sync.dma_start(hgR[0:64, 1:1 + H, 1:1 + W], hin)
            nc.sync.dma_start(hgR[64:128, 0:0 + H, 1:1 + W], hin)
            nc.sync.dma_start(c2[0:64, :], c_in[:])
            nc.sync.dma_start(c2[64:128, :], c_in[:])
            nc.scalar.dma_start(mulc_if[:], mulc_if_in[:])
            nc.scalar.dma_start(mulc_o[:], mulc_o_in[:])
            nc.scalar.dma_start(woff[:], woff_in[:])
            nc.scalar.dma_start(wdcn[:], wdcn_in[:])
            nc.scalar.dma_start(whp[:], whp_in[:])
            nc.scalar.dma_start(boff, boff_in[:])
            nc.scalar.dma_start(consts[:, 1:3], bdcn_in[:])

            # ---- correction head (independent of om/tents) ----
            if n_terms:
                with (
                    tc.tile_pool(name="corrp", bufs=1) as cp,
                    tc.tile_pool(name="psum_c", bufs=1, space="PSUM") as pcs,
                ):
                    ident = cp.tile([128, 128], F16, tag="ident")
                    make_identity(nc, ident[:])
                    wu = cp.tile([128, 5, 256], F16, tag="wu")
                    nc.scalar.dma_start(wu[:], wu_in[:])
                    gidx = cp.tile([128, NC4 // 128], I32, tag="gidx")
                    sidx = cp.tile([128, NUP // 128], I32, tag="sidx")
                    nc.sync.dma_start(gidx[:], gidx_in[:])
                    nc.sync.dma_start(sidx[:], sidx_in[:])
                    xgt = cp.tile([128, NC4], F16, tag="xgt")
                    for ci in range(NC4 // 128):
                        xr = cp.tile([128, C], F16, tag="xr")
                        nc.gpsimd.indirect_dma_start(
                            out=xr[:], out_offset=None,
                            in_=xt16_in[:],
                            in_offset=bass.IndirectOffsetOnAxis(
                                ap=gidx[:, ci:ci + 1], axis=0),
                        )
                        pt = pcs.tile([128, 128], F16, tag="ptx")
                        nc.tensor.transpose(pt[0:64, :], xr[:], ident[:])
                        nc.tensor.transpose(pt[64:128, :], xr[:], ident[:])
                        nc.vector.tensor_copy(
                            xgt[:, ci * 128:(ci + 1) * 128], pt[:])
                    corr_bc = cp.tile([128, NC4], F16, tag="corr_bc")
                    nc.sync.dma_start(
                        corr_bc[0:64, :],
                        corr_in[0:1, :].to_broadcast([64, NC4]))
                    nc.sync.dma_start(
                        corr_bc[64:128, :],
                        corr_in[1:2, :].to_broadcast([64, NC4]))
                    p4 = cp.tile([128, NC4], F16, tag="p4")
                    nc.vector.tensor_mul(p4[:], xgt[:], corr_bc[:])
                    p4v = p4[:].rearrange("p (j s) -> p j s", s=4)
                    ta = cp.tile([128, NUP], F16, tag="ta")
                    tb = cp.tile([128, NUP], F16, tag="tb")
                    prodm = cp.tile([128, NUP], F16, tag="prodm")
                    nc.vector.tensor_add(ta[:], p4v[:, :, 0], p4v[:, :, 1])
                    nc.vector.tensor_add(tb[:], p4v[:, :, 2], p4v[:, :, 3])
                    nc.vector.tensor_add(prodm[:], ta[:], tb[:])

                    psu = [pcs.tile([128, NUP], F32, tag=f"psu{i}",
                                    name=f"psu{i}")
                           for i in range(2)]
                    live = [(g, s, e) for g, (s, e) in enumerate(granges)
                            if e > s]
                    for oh in range(2):
                        for i, (g, s, e) in enumerate(live):
                            nc.tensor.matmul(
                                psu[oh][:, s:e],
                                wu[:, g, oh * 128:(oh + 1) * 128],
                                prodm[:, s:e],
                                start=(i == 0), stop=(i == len(live) - 1))
                    usb = [cp.tile([128, NUP], F16, tag=f"usb{i}",
                                   name=f"usb{i}")
                           for i in range(2)]
                    nc.vector.tensor_copy(usb[0][:], psu[0][:])
                    nc.vector.tensor_copy(usb[1][:], psu[1][:])

                    # zero the DRAM patch using the still-zero PALL0
                    pdz = pd[:].rearrange("(p x) c -> p (x c)", p=128)
                    nc.sync.dma_start(pdz[:, 0:HW], PALL0[:])
                    nc.sync.dma_start(pdz[:, HW:2 * HW], PALL0[:])
                    nc.sync.dma_start(pdz[:, 2 * HW:2 * HW + 2048],
                                      PALL0[:, 0:2048])
                    # transposed U rows -> scatter to pd rows (by pixel)
                    for ci in range(NUP // 128):
                        ut = cp.tile([128, PDC], F16, tag="ut")
                        ptu = pcs.tile([128, 128], F16, tag="ptu")
                        ptv = pcs.tile([128, 128], F16, tag="ptv")
                        nc.tensor.transpose(
                            ptu[:], usb[0][:, ci * 128:(ci + 1) * 128],
                            ident[:])
                        nc.tensor.transpose(
                            ptv[:], usb[1][:, ci * 128:(ci + 1) * 128],
                            ident[:])
                        nc.vector.memset(ut[:, 256:PDC], 0.0)
                        nc.vector.tensor_copy(ut[:, 0:128], ptu[:])
                        nc.vector.tensor_copy(ut[:, 128:256], ptv[:])
                        nc.gpsimd.indirect_dma_start(
                            out=pd[:], out_offset=bass.IndirectOffsetOnAxis(
                                ap=sidx[:, ci:ci + 1], axis=0),
                            in_=ut[:], in_offset=None,
                            bounds_check=HW - 1, oob_is_err=False)
                    # patch maps: if [128], c rows 0:64, o rows 0:64
                    nc.sync.dma_start_transpose(PALL0[:], pd[:, 0:128])
                    nc.sync.dma_start_transpose(PCOc[:], pd[:, 128:256])
                    nc.sync.dma_start_transpose(PCOo[:], pd[:, 192:PDC])
                    for (j, p) in ov:
                        nc.vector.tensor_add(PALL0[:, p:p + 1],
                                             PALL0[:, p:p + 1],
                                             usb[0][:, j:j + 1])
                        nc.vector.tensor_add(PCOc[0:64, p:p + 1],
                                             PCOc[0:64, p:p + 1],
                                             usb[1][0:64, j:j + 1])
                        tmp1 = cp.tile([64, 1], F16, tag="ovt")
                        nc.sync.dma_start(tmp1[:], usb[1][64:128, j:j + 1])
                        nc.vector.tensor_add(PCOo[0:64, p:p + 1],
                                             PCOo[0:64, p:p + 1], tmp1[:])

            # ---- offset conv + tents + psi products ----
            with tc.tile_pool(name="maps", bufs=1) as mp:
                dy36 = mp.tile([36, 1024], F32, tag="dy36")
                dx36 = mp.tile([36, 1024], F32, tag="dx36")
                msk = mp.tile([36, 1024], F32, tag="msk")
                tY = [mp.tile([36, 1024], F32, tag=f"tY{u}", name=f"tY{u}")
                      for u in range(5)]
                tX = [mp.tile([36, 1024], F32, tag=f"tX{u}", name=f"tX{u}")
                      for u in range(5)]

                with (
                    tc.tile_pool(name="omp", bufs=2) as omp_,
                    tc.tile_pool(name="psum_om", bufs=2, space="PSUM") as psom,
                ):
                    for q in range(4):
                        omq = omp_.tile([27, 1024], F32, tag="om_q")
                        for hb in range(2):
                            blk = q * 2 + hb
                            ps = psom.tile([27, 512], F32, tag="omps")
                            for t in range(KK):
                                ky, kx = t // 3, t % 3
                                rhs = xg["C"][0:64,
                                              blk * 8 + ky + 2:
                                              blk * 8 + ky + 10,
                                              kx + 2:kx + 2 + W]
                                nc.tensor.matmul(ps[:], woff[:, t, :], rhs,
                                                 start=(t == 0),
                                                 stop=(t == KK - 1))
                            nc.scalar.activation(
                                omq[:, hb * 512:(hb + 1) * 512], ps[:],
                                AF.Identity, bias=boff, scale=1.0)
                        nc.sync.dma_start(
                            om_dram[:, q * 1024:(q + 1) * 1024], omq[:])
                    for (dst, r0) in ((dy36, 0), (dx36, 9), (msk, 18)):
                        nc.sync.dma_start(
                            dst[:],
                            om_dram[r0:r0 + 9, :].rearrange(
                                "p (q f) -> (p q) f", q=4))

                nc.scalar.activation(msk[:], msk[:], AF.Sigmoid)
                for (src, tT) in ((dy36, tY), (dx36, tX)):
                    nc.scalar.activation(tT[3][:], src[:], AF.Relu)
                    nc.scalar.activation(tT[1][:], src[:], AF.Relu,
                                         scale=-1.0)
                    nc.scalar.activation(tT[4][:], src[:], AF.Relu,
                                         bias=consts[0:36, 3:4])
                    nc.scalar.activation(tT[0][:], src[:], AF.Relu,
                                         scale=-1.0, bias=consts[0:36, 3:4])
                    nc.vector.tensor_add(tT[2][:], tT[3][:], tT[1][:])
                    nc.scalar.activation(tT[2][:], tT[2][:], AF.Relu,
                                         scale=-1.0, bias=1.0)
                    nc.vector.scalar_tensor_tensor(tT[3][:], tT[4][:], -2.0,
                                                   tT[3][:], OP.mult, OP.add)
                    nc.vector.scalar_tensor_tensor(tT[1][:], tT[0][:], -2.0,
                                                   tT[1][:], OP.mult, OP.add)
                for ub in (1, 2, 3):
                    nc.vector.tensor_mul(tY[ub][:], tY[ub][:], msk[:])

                psi16 = mp.tile([36, 1024], F16, tag="psi16")
                for ub in (1, 2, 3):
                    for vb in (1, 2, 3):
                        nc.vector.tensor_mul(psi16[:], tY[ub][:], tX[vb][:])
                        r0 = ((ub - 1) * 3 + (vb - 1)) * KK
                        nc.sync.dma_start(
                            psi_dram[r0:r0 + KK, :].rearrange(
                                "p (q f) -> (p q) f", q=4),
                            psi16[:])

            # ---- fused sampling + DCN + h-conv + gates ----
            bc_engines = [nc.sync, nc.gpsimd, nc.scalar]
            bc_state = [0]

            def bc_dma(dst, src):
                bc_engines[bc_state[0] % 3].dma_start(dst, src)
                bc_state[0] += 1

            hops = [("C", 0, 0, True), ("C", 1, 0, True), ("C", 2, 0, True),
                    ("R", 0, 2, True), ("C", 2, 2, False)]

            with (
                tc.tile_pool(name="bcp", bufs=6) as bcp,
                tc.tile_pool(name="bcs", bufs=2) as bcs,
                tc.tile_pool(name="rp", bufs=10) as rp,
                tc.tile_pool(name="psum_g", bufs=8, space="PSUM") as psg,
                tc.tile_pool(name="gw", bufs=1) as gw,
            ):
                for gp in range(2):
                    glo, ghi = gp * 2048, (gp + 1) * 2048
                    ps = [[psg.tile([128, 512], F32, tag="psb",
                                    name=f"ps{gp}_{b}_{o}")
                           for o in range(2)] for b in range(4)]
                    for k in range(KK):
                        kops = _k_ops(k)
                        bct = []
                        for (pair, xs, r0, c0, rl, rh) in kops:
                            if pair:
                                t = bcp.tile([128, 2048], F16, tag="bcp128",
                                             name="bcpair")
                            else:
                                t = bcs.tile([64, 2048], F16, tag="bcs64",
                                             name="bcsolo")
                            bc_dma(t[0:64, :],
                                   psi_dram[rl:rl + 1, glo:ghi].to_broadcast(
                                       [64, 2048]))
                            if pair:
                                bc_dma(t[64:128, :],
                                       psi_dram[rh:rh + 1, glo:ghi]
                                       .to_broadcast([64, 2048]))
                            bct.append(t)
                        for gh in range(2):
                            G = gp * 2 + gh
                            rts = []
                            for oi, (pair, xs, r0, c0, rl, rh) in \
                                    enumerate(kops):
                                sel = xs if c0 % 2 == 0 else xs + "b"
                                cc = c0 if c0 % 2 == 0 else c0 - 1
                                np_ = 128 if pair else 64
                                rt = rp.tile([128, 1024], F16, tag="rt")
                                if not pair:
                                    nc.vector.memset(rt[64:128, :], 0.0)
                                nc.vector.tensor_mul(
                                    rt[0:np_, :].rearrange(
                                        "p (r c) -> p r c", c=W),
                                    bct[oi][0:np_, gh * 1024:(gh + 1) * 1024]
                                    .rearrange("p (r c) -> p r c", c=W),
                                    xg[sel][0:np_,
                                            G * 16 + r0:G * 16 + r0 + 16,
                                            cc:cc + W])
                                rts.append(rt)
                            for oh in range(2):
                                st = wdcn[:, k, oh * 128:(oh + 1) * 128]
                                for oi, rt in enumerate(rts):
                                    for cb in range(2):
                                        nc.tensor.matmul(
                                            ps[gh * 2 + cb][oh][:], st,
                                            rt[:, cb * 512:(cb + 1) * 512],
                                            start=(k == 0 and oi == 0),
                                            stop=False)

                    # h-conv for the 4 blocks of this group-pair
                    for oh in range(2):
                        for j, (hs, ky, kx, pair) in enumerate(hops):
                            hv = hgC if hs == "C" else hgR
                            np_ = 128 if pair else 64
                            st = whp[0:np_, j, oh * 128:(oh + 1) * 128]
                            for bl in range(4):
                                blk = gp * 4 + bl
                                nc.tensor.matmul(
                                    ps[bl][oh][:], st,
                                    hv[0:np_, blk * 8 + ky:blk * 8 + ky + 8,
                                       kx:kx + W],
                                    start=False, stop=(j == len(hops) - 1))

                    # gates per block
                    for bl in range(4):
                        blk = gp * 4 + bl
                        lo, hi = blk * 512, (blk + 1) * 512
                        ps0, ps1 = ps[bl][0], ps[bl][1]
                        tif = gw.tile([128, 512], F32, tag="tif")
                        uif = gw.tile([128, 512], F32, tag="uif")
                        uif2 = gw.tile([128, 512], F32, tag="uif2")
                        ift = gw.tile([128, 512], F32, tag="ift")
                        cgc = gw.tile([128, 512], F32, tag="cgc")
                        tc_ = gw.tile([64, 512], F32, tag="tc_")
                        prod = gw.tile([64, 512], F32, tag="prod")
                        pf = gw.tile([64, 512], F32, tag="pf")
                        cnx = gw.tile([64, 512], F32, tag="cnx")
                        to_ = gw.tile([64, 512], F32, tag="to")
                        to2 = gw.tile([64, 512], F32, tag="to2")
                        uo = gw.tile([64, 512], F32, tag="uo")
                        ot = gw.tile([64, 512], F32, tag="ot")
                        rc = gw.tile([64, 512], F32, tag="rc")
                        hnx = gw.tile([64, 512], F32, tag="hnx")

                        nc.vector.tensor_mul(tif[:], mulc_if[:, lo:hi],
                                             c2[:, lo:hi])
                        nc.vector.scalar_tensor_tensor(
                            uif[:], ps0[:], 1.0, tif[:], OP.mult, OP.add)
                        nc.vector.tensor_add(uif2[:], uif[:], PALL0[:, lo:hi])
                        nc.scalar.activation(ift[:], uif2[:], AF.Sigmoid,
                                             bias=bdcn0)
                        nc.vector.tensor_add(tc_[:], ps1[0:64, :],
                                             PCOc[0:64, lo:hi])
                        nc.scalar.activation(cgc[0:64, :], tc_[:], AF.Relu,
                                             bias=consts[0:64, 2:3])
                        nc.scalar.activation(cgc[64:128, :],
                                             c2[64:128, lo:hi], AF.Copy)
                        nc.vector.tensor_mul(prod[:], ift[0:64, :],
                                             cgc[0:64, :])
                        nc.vector.tensor_mul(pf[:], ift[64:128, :],
                                             cgc[64:128, :])
                        nc.vector.tensor_add(cnx[:], prod[:], pf[:])
                        nc.vector.tensor_mul(to_[:], mulc_o[:, lo:hi],
                                             cnx[:])
                        nc.vector.tensor_add(to2[:], to_[:],
                                             PCOo[0:64, lo:hi])
                        nc.vector.scalar_tensor_tensor(
                            uo[:], ps1[64:128, :], 1.0, to2[:],
                            OP.mult, OP.add)
                        nc.scalar.activation(ot[:], uo[:], AF.Sigmoid,
                                             bias=consts[64:128, 2:3])
                        nc.scalar.activation(rc[:], cnx[:], AF.Relu)
                        nc.vector.tensor_mul(hnx[:], ot[:], rc[:])
                        nc.sync.dma_start(c_out[:, lo:hi], cnx[:])
                        nc.sync.dma_start(h_out[:, lo:hi], hnx[:])

    nc.compile()
    return nc


# ---------------- host side ----------------

def _host_om(x, w_off, b_off):
    x = np.asarray(x, np.float32)
    w = np.asarray(w_off, np.float32)
    bb = np.asarray(b_off, np.float32)
    xp = np.pad(x, ((0, 0), (0, 0), (1, 1), (1, 1)))
    om = np.zeros((B, 3 * KK, H, W), np.float32)
    for ky in range(3):
        for kx in range(3):
            om += np.einsum("oc,bchw->bohw", w[:, :, ky, kx],
                            xp[:, :, ky:ky + H, kx:kx + W], optimize=True)
    return om + bb[None, :, None, None]


def _tent(d, j):
    a1 = max(d, 0.0)
    a2 = max(d - 1.0, 0.0)
    b1 = max(-d, 0.0)
    b2 = max(-d - 1.0, 0.0)
    return (b2, b1 - 2 * b2, max(1.0 - a1 - b1, 0.0), a1 - 2 * a2, a2)[j]


def compute_corr(x, w_off, b_off):
    """Union correction structure + per-core coefficient values."""
    om = _host_om(x, w_off, b_off)
    dy, dx = om[:, :KK], om[:, KK:2 * KK]
    mask = 1.0 / (1.0 + np.exp(-om[:, 2 * KK:]))
    PR = 2e-3

    uniq = {}   # (g, p) -> slot list [(k, u, v, nbr)]
    vals = {}   # (b, k, u, v, p) -> value
    viol = (np.abs(dy) > 1.0) | (np.abs(dx) > 1.0)
    bidx, kidx, ridx, widx = np.nonzero(viol)
    for b, k, r, c in zip(bidx, kidx, ridx, widx):
        kh, kw = k // 3 - 1, k % 3 - 1
        dyv = float(dy[b, k, r, c])
        dxv = float(dx[b, k, r, c])
        mv = float(mask[b, k, r, c])
        assert abs(dyv) < 2.0 and abs(dxv) < 2.0, "offset >= 2 unsupported"
        p = int(r) * W + int(c)
        for ju in range(5):
            for jv in range(5):
                u, v = ju - 2, jv - 2
                if abs(u) != 2 and abs(v) != 2:
                    continue
                val = _tent(dyv, ju) * _tent(dxv, jv) * mv
                if abs(val) < PR:
                    continue
                nr, ncol = int(r) + kh + u, int(c) + kw + v
                if not (0 <= nr < H and 0 <= ncol < W):
                    continue
                slots = uniq.setdefault((k // 2, p), [])
                sk = (int(k), u, v, nr * W + ncol)
                if sk not in slots:
                    slots.append(sk)
                vals[(int(b), int(k), u, v, p)] = val

    normal, ovl = [], []   # (g, p, slots<=4)
    seen = set()
    for (g, p), slots in sorted(uniq.items()):
        chunks = [slots[i:i + 4] for i in range(0, len(slots), 4)]
        for ci, chunk in enumerate(chunks):
            if ci == 0 and p not in seen:
                normal.append((g, p, chunk))
                seen.add(p)
            else:
                ovl.append((g, p, chunk))

    bygroup = [[] for _ in range(5)]
    for (g, p, slots) in normal:
        bygroup[g].append((p, slots, False))
    for (g, p, slots) in ovl:
        bygroup[g].append((p, slots, True))

    granges = []
    gidx = np.zeros(NC4, np.int64)
    sidx = np.full(NUP, HW, np.int64)   # HW -> skipped by bounds check
    ov = []
    colinfo = []
    j = 0
    for g in range(5):
        s = j
        for (p, slots, is_ov) in bygroup[g]:
            assert j < NUP, "too many correction columns"
            for si, (k, u, v, nbr) in enumerate(slots):
                gidx[4 * j + si] = nbr
            if is_ov:
                ov.append((j, p))
            else:
                sidx[j] = p
            colinfo.append((j, p, slots))
            j += 1
        granges.append([s, j])
    granges[4][1] = NUP  # pad columns keep PSUM fully written
    assert len(ov) <= 8, f"too many overflow columns: {len(ov)}"

    corr4 = np.zeros((B, 2, NC4), np.float32)
    for (jj, p, slots) in colinfo:
        for si, (k, u, v, nbr) in enumerate(slots):
            half = k % 2
            for b in range(B):
                val = vals.get((b, k, u, v, p))
                if val:
                    corr4[b, half, 4 * jj + si] = val

    gidx128 = np.zeros((128, NC4 // 128), np.int32)
    for i in range(NC4):
        gidx128[i % 128, i // 128] = gidx[i]
    sidx128 = np.zeros((128, NUP // 128), np.int32)
    for i in range(NUP):
        sidx128[i % 128, i // 128] = sidx[i]
    return (tuple(tuple(r) for r in granges), tuple(ov), gidx128, sidx128,
            corr4.astype(np.float16))


def make_in_maps(x, h, c, w_off, b_off, w_dcn, b_dcn, w_h, mul_c,
                 gidx128, sidx128, corr4):
    x = np.ascontiguousarray(x, np.float32)
    h = np.ascontiguousarray(h, np.float32)
    c = np.ascontiguousarray(c, np.float32)
    mul_c = np.asarray(mul_c, np.float32)

    mulc_if = np.ascontiguousarray(
        mul_c[0, 0:128].reshape(128, HW)).astype(np.float16)
    mulc_o = np.ascontiguousarray(
        mul_c[0, 128:192].reshape(64, HW)).astype(np.float16)
    woff = np.ascontiguousarray(
        np.asarray(w_off, np.float32).reshape(27, 64, KK)
        .transpose(1, 2, 0)).astype(np.float16)
    boff = np.asarray(b_off, np.float32).reshape(27, 1)
    wd = np.asarray(w_dcn, np.float32).reshape(256, 64, KK)
    wdcnp = np.zeros((128, KK, 256), np.float32)
    for k in range(KK):
        wk = wd[:, :, k].T
        wdcnp[0:64, k] = wk
        wdcnp[64:128, k] = wk
    wu = np.zeros((128, 5, 256), np.float32)
    for g in range(5):
        wu[0:64, g] = wd[:, :, 2 * g].T
        if 2 * g + 1 < KK:
            wu[64:128, g] = wd[:, :, 2 * g + 1].T
    bdcn = np.ascontiguousarray(
        np.asarray(b_dcn, np.float32).reshape(2, 128).T)
    wh = np.asarray(w_h, np.float32).reshape(256, 64, KK)
    whp = np.zeros((128, 5, 256), np.float32)
    for j, (ta, tb) in enumerate(((0, 1), (3, 4), (6, 7), (2, 5))):
        whp[0:64, j] = wh[:, :, ta].T
        whp[64:128, j] = wh[:, :, tb].T
    whp[0:64, 4] = wh[:, :, 8].T

    shared = dict(mulc_if=mulc_if, mulc_o=mulc_o, woff=woff, boff=boff,
                  wdcnp=wdcnp.astype(np.float16), wu=wu.astype(np.float16),
                  bdcn=bdcn, whp=whp.astype(np.float16),
                  gidx=gidx128, sidx=sidx128)
    in_maps = []
    for b in range(B):
        m = dict(shared)
        x16 = x[b].reshape(C, HW).astype(np.float16)
        m["x16"] = x16
        m["xt16"] = np.ascontiguousarray(x16.T)
        m["h16"] = h[b].reshape(C, HW).astype(np.float16)
        m["cf"] = c[b].reshape(C, HW)
        m["corr4"] = corr4[b].astype(np.float16)
        in_maps.append(m)
    return in_maps


def get_nc(granges, ov):
    key = (granges, ov)
    if key not in _COMPILED:
        _COMPILED[key] = _build(granges, ov)
    return _COMPILED[key]


def kernel(x, h, c, w_off, b_off, w_dcn, b_dcn, w_h, mul_c):
    granges, ov, gidx128, sidx128, corr4 = compute_corr(x, w_off, b_off)
    nc = get_nc(granges, ov)
    in_maps = make_in_maps(x, h, c, w_off, b_off, w_dcn, b_dcn, w_h, mul_c,
                           gidx128, sidx128, corr4)
    res = run_bass_kernel_spmd(nc, in_maps, core_ids=list(range(B)))
    h_next = np.stack([res.results[b]["h_out"].reshape(C, H, W)
                       for b in range(B)])
    c_next = np.stack([res.results[b]["c_out"].reshape(C, H, W)
                       for b in range(B)])
    return h_next.astype(np.float32), c_next.astype(np.float32)


# revision 12
# speedup vs baseline: 3.0016x; 1.1052x over previous
"""Trainium2 Bass kernel for DCN_ConvLSTM2D (v2 — fused matmul pipeline).

Math (per batch element, data-parallel over 8 cores):
  om    = conv3x3(x, w_off) + b_off            -> dy, dx, mask=sigmoid
  x_cat = modulated deformable conv (DCNv2)
  h_cat = conv3x3(h, w_h)
  LSTM gates with peephole mul_c; outputs (h_next, c_next).

v2 design (vs the S-materializing baseline):
  * Bilinear sampling via the exact tent window. Dense taps
    (u,v) in {-1,0,1}^2: DVE products R = psi_bc * x_window stream
    directly into the DCN matmul accumulation (PSUM sums the taps) —
    no S tensor and no DVE adds.
  * Tap pairing: x lives in [128, 70*70] tiles whose upper 64
    partitions hold the image shifted by +1 col (xpC) or +1 row (xpR),
    plus 1-element-shifted b-variants keeping fp16 DVE reads 4B
    aligned. One [128,*] DVE product covers TWO taps; the stationary
    (w_k; w_k) contracts both halves: 5 operands/k instead of 9.
  * All matmul paths fp16 (om conv, h conv, DCN).
  * |offset|>1 tail taps: host evaluates their tent coefficients
    (indices AND values; ~1e-3 off device numerics, far inside the
    2e-2 gate). Device gathers x-neighbor columns with indirect DMA,
    contracts them against w_dcn in ~10 small matmuls, scatters
    per-pixel patch rows to DRAM with 2 indirect DMAs, and
    transpose-loads patch maps that the gate stage adds in 3 wide DVE
    ops per block. Other cores' terms carry zero coefficients
    (SPMD-safe union).
"""

import numpy as np
import ml_dtypes

import concourse.bacc as bacc
import concourse.bass as bass
import concourse.mybir as mybir
import concourse.tile as tile
from concourse.bass_utils import run_bass_kernel_spmd
from concourse.masks import make_identity

F32 = mybir.dt.float32
BF16 = mybir.dt.bfloat16
I32 = mybir.dt.int32
AF = mybir.ActivationFunctionType
OP = mybir.AluOpType

B, C, H, W = 8, 64, 64, 64
HW = H * W
KK = 9
XP = 70    # x padded grid (pad 3)
HP = 66    # h padded grid (pad 1)
NUP = 256  # padded correction-column count
NC4 = 4 * NUP
PDC = 320  # patch DRAM row width: 128 (if) + 64 (c) + 128 (o at cols 192:256 + pad)

_COMPILED = {}


def _row(k, u, v):
    return ((u + 1) * 3 + (v + 1)) * KK + k


def _k_ops(k):
    """Dense-tap operand table: (pair?, xsel, r0, c0, row_lo, row_hi)."""
    kh, kw = k // 3 - 1, k % 3 - 1
    ops = []
    for u in (-1, 0, 1):  # v-pair ((u,-1) lower, (u,0) upper) via xpC
        ops.append((True, "C", 3 + kh + u, 2 + kw,
                    _row(k, u, -1), _row(k, u, 0)))
    # u-pair ((-1,+1) lower, (0,+1) upper) via xpR
    ops.append((True, "R", 2 + kh, 4 + kw, _row(k, -1, 1), _row(k, 0, 1)))
    # solo (+1,+1): lower 64 partitions, upper zeroed
    ops.append((False, "C", 4 + kh, 4 + kw, _row(k, 1, 1), None))
    return ops


def _build(granges, ov):
    nc = bacc.Bacc(None, target_bir_lowering=False)

    xt16_in = nc.dram_tensor("xt16", [HW, C], BF16, kind="ExternalInput")
    xpc_in = nc.dram_tensor("xpc", [128, XP * XP], BF16, kind="ExternalInput")
    xpcb_in = nc.dram_tensor("xpcb", [128, XP * XP], BF16, kind="ExternalInput")
    xpr_in = nc.dram_tensor("xpr", [128, XP * XP], BF16, kind="ExternalInput")
    xprb_in = nc.dram_tensor("xprb", [128, XP * XP], BF16, kind="ExternalInput")
    hpc_in = nc.dram_tensor("hpc", [128, HP * HP], BF16, kind="ExternalInput")
    hpr_in = nc.dram_tensor("hpr", [128, HP * HP], BF16, kind="ExternalInput")
    c_in = nc.dram_tensor("cf", [C, HW], F32, kind="ExternalInput")
    mulc_if_in = nc.dram_tensor("mulc_if", [128, HW], BF16, kind="ExternalInput")
    mulc_o_in = nc.dram_tensor("mulc_o", [64, HW], BF16, kind="ExternalInput")
    woff_in = nc.dram_tensor("woff", [64, KK, 27], BF16, kind="ExternalInput")
    boff_in = nc.dram_tensor("boff", [27, 1], F32, kind="ExternalInput")
    wdcn_in = nc.dram_tensor("wdcnp", [128, KK, 256], BF16, kind="ExternalInput")
    wu_in = nc.dram_tensor("wu", [128, 5, 256], BF16, kind="ExternalInput")
    bdcn_in = nc.dram_tensor("bdcn", [128, 2], F32, kind="ExternalInput")
    whp_in = nc.dram_tensor("whp", [128, 5, 256], BF16, kind="ExternalInput")
    corr_in = nc.dram_tensor("corr4", [2, NC4], BF16, kind="ExternalInput")
    gidx_in = nc.dram_tensor("gidx", [128, NC4 // 128], I32, kind="ExternalInput")
    sidx_in = nc.dram_tensor("sidx", [128, NUP // 128], I32, kind="ExternalInput")

    om_dram = nc.dram_tensor("om_scratch", [27, HW], F32)
    psi_dram = nc.dram_tensor("psi_scratch", [81, HW], BF16)
    pd = nc.dram_tensor("patch_scratch", [HW, PDC], BF16)

    h_out = nc.dram_tensor("h_out", [C, HW], F32, kind="ExternalOutput")
    c_out = nc.dram_tensor("c_out", [C, HW], F32, kind="ExternalOutput")

    n_terms = sum(e - s for s, e in granges)

    with tile.TileContext(nc) as tc:
        with tc.tile_pool(name="persist", bufs=1) as pp:
            xpC = pp.tile([128, XP * XP], BF16, tag="xpC")
            xpCb = pp.tile([128, XP * XP], BF16, tag="xpCb")
            xpR = pp.tile([128, XP * XP], BF16, tag="xpR")
            xpRb = pp.tile([128, XP * XP], BF16, tag="xpRb")
            hpC = pp.tile([128, HP * HP], BF16, tag="hpC")
            hpR = pp.tile([128, HP * HP], BF16, tag="hpR")
            c2 = pp.tile([128, HW], F32, tag="c2")
            mulc_if = pp.tile([128, HW], BF16, tag="mulc_if")
            mulc_o = pp.tile([64, HW], BF16, tag="mulc_o")
            woff = pp.tile([64, KK, 27], BF16, tag="woff")
            wdcn = pp.tile([128, KK, 256], BF16, tag="wdcn")
            whp = pp.tile([128, 5, 256], BF16, tag="whp")
            consts = pp.tile([128, 4], F32, tag="consts")
            PALL0 = pp.tile([128, HW], BF16, tag="PALL0")   # if-patch
            PCOc = pp.tile([128, HW], BF16, tag="PCOc")     # rows 0:64 = c-patch
            PCOo = pp.tile([128, HW], BF16, tag="PCOo")     # rows 0:64 = o-patch
            boff = consts[0:27, 0:1]
            bdcn0 = consts[:, 1:2]

            nc.vector.memset(PALL0[:], 0.0)
            nc.vector.memset(PCOc[:], 0.0)
            nc.vector.memset(PCOo[:], 0.0)
            nc.vector.memset(consts[:, 3:4], -1.0)

            xg = {n: t[:].rearrange("p (r c) -> p r c", c=XP)
                  for n, t in (("C", xpC), ("Cb", xpCb),
                               ("R", xpR), ("Rb", xpRb))}
            hgC = hpC[:].rearrange("p (r c) -> p r c", c=HP)
            hgR = hpR[:].rearrange("p (r c) -> p r c", c=HP)

            nc.sync.dma_start(xpC[:], xpc_in[:])
            nc.sync.dma_start(xpCb[:], xpcb_in[:])
            nc.sync.dma_start(xpR[:], xpr_in[:])
            nc.sync.dma_start(xpRb[:], xprb_in[:])
            nc.sync.dma_start(hpC[:], hpc_in[:])
            nc.sync.dma_start(hpR[:], hpr_in[:])
            nc.sync.dma_start(c2[0:64, :], c_in[:])
            nc.sync.dma_start(c2[64:128, :], c_in[:])
            nc.scalar.dma_start(mulc_if[:], mulc_if_in[:])
            nc.scalar.dma_start(mulc_o[:], mulc_o_in[:])
            nc.scalar.dma_start(woff[:], woff_in[:])
            nc.scalar.dma_start(wdcn[:], wdcn_in[:])
            nc.scalar.dma_start(whp[:], whp_in[:])
            nc.scalar.dma_start(boff, boff_in[:])
            nc.scalar.dma_start(consts[:, 1:3], bdcn_in[:])

            # ---- correction head (independent of om/tents) ----
            if n_terms:
                with (
                    tc.tile_pool(name="corrp", bufs=1) as cp,
                    tc.tile_pool(name="psum_c", bufs=1, space="PSUM") as pcs,
                ):
                    ident = cp.tile([128, 128], BF16, tag="ident")
                    make_identity(nc, ident[:])
                    wu = cp.tile([128, 5, 256], BF16, tag="wu")
                    nc.scalar.dma_start(wu[:], wu_in[:])
                    gidx = cp.tile([128, NC4 // 128], I32, tag="gidx")
                    sidx = cp.tile([128, NUP // 128], I32, tag="sidx")
                    nc.sync.dma_start(gidx[:], gidx_in[:])
                    nc.sync.dma_start(sidx[:], sidx_in[:])
                    xgt = cp.tile([128, NC4], BF16, tag="xgt")
                    for ci in range(NC4 // 128):
                        xr = cp.tile([128, C], BF16, tag="xr")
                        nc.gpsimd.indirect_dma_start(
                            out=xr[:], out_offset=None,
                            in_=xt16_in[:],
                            in_offset=bass.IndirectOffsetOnAxis(
                                ap=gidx[:, ci:ci + 1], axis=0),
                        )
                        pt = pcs.tile([128, 128], BF16, tag="ptx")
                        nc.tensor.transpose(pt[0:64, :], xr[:], ident[:])
                        nc.tensor.transpose(pt[64:128, :], xr[:], ident[:])
                        nc.vector.tensor_copy(
                            xgt[:, ci * 128:(ci + 1) * 128], pt[:])
                    corr_bc = cp.tile([128, NC4], BF16, tag="corr_bc")
                    nc.sync.dma_start(
                        corr_bc[0:64, :],
                        corr_in[0:1, :].to_broadcast([64, NC4]))
                    nc.sync.dma_start(
                        corr_bc[64:128, :],
                        corr_in[1:2, :].to_broadcast([64, NC4]))
                    p4 = cp.tile([128, NC4], BF16, tag="p4")
                    nc.vector.tensor_mul(p4[:], xgt[:], corr_bc[:])
                    p4v = p4[:].rearrange("p (j s) -> p j s", s=4)
                    ta = cp.tile([128, NUP], BF16, tag="ta")
                    tb = cp.tile([128, NUP], BF16, tag="tb")
                    prodm = cp.tile([128, NUP], BF16, tag="prodm")
                    nc.vector.tensor_add(ta[:], p4v[:, :, 0], p4v[:, :, 1])
                    nc.vector.tensor_add(tb[:], p4v[:, :, 2], p4v[:, :, 3])
                    nc.vector.tensor_add(prodm[:], ta[:], tb[:])

                    psu = [pcs.tile([128, NUP], F32, tag=f"psu{i}",
                                    name=f"psu{i}")
                           for i in range(2)]
                    live = [(g, s, e) for g, (s, e) in enumerate(granges)
                            if e > s]
                    for oh in range(2):
                        for i, (g, s, e) in enumerate(live):
                            nc.tensor.matmul(
                                psu[oh][:, s:e],
                                wu[:, g, oh * 128:(oh + 1) * 128],
                                prodm[:, s:e],
                                start=(i == 0), stop=(i == len(live) - 1))
                    usb = [cp.tile([128, NUP], BF16, tag=f"usb{i}",
                                   name=f"usb{i}")
                           for i in range(2)]
                    nc.vector.tensor_copy(usb[0][:], psu[0][:])
                    nc.vector.tensor_copy(usb[1][:], psu[1][:])

                    # zero the DRAM patch using the still-zero PALL0
                    pdz = pd[:].rearrange("(p x) c -> p (x c)", p=128)
                    nc.sync.dma_start(pdz[:, 0:HW], PALL0[:])
                    nc.sync.dma_start(pdz[:, HW:2 * HW], PALL0[:])
                    nc.sync.dma_start(pdz[:, 2 * HW:2 * HW + 2048],
                                      PALL0[:, 0:2048])
                    # transposed U rows -> scatter to pd rows (by pixel)
                    for ci in range(NUP // 128):
                        ut = cp.tile([128, PDC], BF16, tag="ut")
                        ptu = pcs.tile([128, 128], BF16, tag="ptu")
                        ptv = pcs.tile([128, 128], BF16, tag="ptv")
                        nc.tensor.transpose(
                            ptu[:], usb[0][:, ci * 128:(ci + 1) * 128],
                            ident[:])
                        nc.tensor.transpose(
                            ptv[:], usb[1][:, ci * 128:(ci + 1) * 128],
                            ident[:])
                        nc.vector.memset(ut[:, 256:PDC], 0.0)
                        nc.vector.tensor_copy(ut[:, 0:128], ptu[:])
                        nc.vector.tensor_copy(ut[:, 128:256], ptv[:])
                        nc.gpsimd.indirect_dma_start(
                            out=pd[:], out_offset=bass.IndirectOffsetOnAxis(
                                ap=sidx[:, ci:ci + 1], axis=0),
                            in_=ut[:], in_offset=None,
                            bounds_check=HW - 1, oob_is_err=False)
                    # patch maps: if [128], c rows 0:64, o rows 0:64
                    nc.sync.dma_start_transpose(PALL0[:], pd[:, 0:128])
                    nc.sync.dma_start_transpose(PCOc[:], pd[:, 128:256])
                    nc.sync.dma_start_transpose(PCOo[:], pd[:, 192:PDC])
                    for (j, p) in ov:
                        nc.vector.tensor_add(PALL0[:, p:p + 1],
                                             PALL0[:, p:p + 1],
                                             usb[0][:, j:j + 1])
                        nc.vector.tensor_add(PCOc[0:64, p:p + 1],
                                             PCOc[0:64, p:p + 1],
                                             usb[1][0:64, j:j + 1])
                        tmp1 = cp.tile([64, 1], BF16, tag="ovt")
                        nc.sync.dma_start(tmp1[:], usb[1][64:128, j:j + 1])
                        nc.vector.tensor_add(PCOo[0:64, p:p + 1],
                                             PCOo[0:64, p:p + 1], tmp1[:])

            # ---- offset conv + tents + psi products ----
            with tc.tile_pool(name="maps", bufs=1) as mp:
                dy36 = mp.tile([36, 1024], F32, tag="dy36")
                dx36 = mp.tile([36, 1024], F32, tag="dx36")
                msk = mp.tile([36, 1024], F32, tag="msk")
                tY = [mp.tile([36, 1024], F32, tag=f"tY{u}", name=f"tY{u}")
                      for u in range(5)]
                tX = [mp.tile([36, 1024], F32, tag=f"tX{u}", name=f"tX{u}")
                      for u in range(5)]

                with (
                    tc.tile_pool(name="omp", bufs=2) as omp_,
                    tc.tile_pool(name="psum_om", bufs=2, space="PSUM") as psom,
                ):
                    for q in range(4):
                        omq = omp_.tile([27, 1024], F32, tag="om_q")
                        for hb in range(2):
                            blk = q * 2 + hb
                            ps = psom.tile([27, 512], F32, tag="omps")
                            for t in range(KK):
                                ky, kx = t // 3, t % 3
                                rhs = xg["C"][0:64,
                                              blk * 8 + ky + 2:
                                              blk * 8 + ky + 10,
                                              kx + 2:kx + 2 + W]
                                nc.tensor.matmul(ps[:], woff[:, t, :], rhs,
                                                 start=(t == 0),
                                                 stop=(t == KK - 1))
                            nc.scalar.activation(
                                omq[:, hb * 512:(hb + 1) * 512], ps[:],
                                AF.Identity, bias=boff, scale=1.0)
                        nc.sync.dma_start(
                            om_dram[:, q * 1024:(q + 1) * 1024], omq[:])
                    for (dst, r0) in ((dy36, 0), (dx36, 9), (msk, 18)):
                        nc.sync.dma_start(
                            dst[:],
                            om_dram[r0:r0 + 9, :].rearrange(
                                "p (q f) -> (p q) f", q=4))

                nc.scalar.activation(msk[:], msk[:], AF.Sigmoid)
                for (src, tT) in ((dy36, tY), (dx36, tX)):
                    nc.scalar.activation(tT[3][:], src[:], AF.Relu)
                    nc.scalar.activation(tT[1][:], src[:], AF.Relu,
                                         scale=-1.0)
                    nc.scalar.activation(tT[4][:], src[:], AF.Relu,
                                         bias=consts[0:36, 3:4])
                    nc.scalar.activation(tT[0][:], src[:], AF.Relu,
                                         scale=-1.0, bias=consts[0:36, 3:4])
                    nc.vector.tensor_add(tT[2][:], tT[3][:], tT[1][:])
                    nc.scalar.activation(tT[2][:], tT[2][:], AF.Relu,
                                         scale=-1.0, bias=1.0)
                    nc.vector.scalar_tensor_tensor(tT[3][:], tT[4][:], -2.0,
                                                   tT[3][:], OP.mult, OP.add)
                    nc.vector.scalar_tensor_tensor(tT[1][:], tT[0][:], -2.0,
                                                   tT[1][:], OP.mult, OP.add)
                for ub in (1, 2, 3):
                    nc.vector.tensor_mul(tY[ub][:], tY[ub][:], msk[:])

                psi16 = mp.tile([36, 1024], BF16, tag="psi16")
                for ub in (1, 2, 3):
                    for vb in (1, 2, 3):
                        nc.vector.tensor_mul(psi16[:], tY[ub][:], tX[vb][:])
                        r0 = ((ub - 1) * 3 + (vb - 1)) * KK
                        nc.sync.dma_start(
                            psi_dram[r0:r0 + KK, :].rearrange(
                                "p (q f) -> (p q) f", q=4),
                            psi16[:])

            # ---- fused sampling + DCN + h-conv + gates ----
            bc_engines = [nc.sync, nc.gpsimd, nc.scalar]
            bc_state = [0]

            def bc_dma(dst, src):
                bc_engines[bc_state[0] % 3].dma_start(dst, src)
                bc_state[0] += 1

            hops = [("C", 0, 0, True), ("C", 1, 0, True), ("C", 2, 0, True),
                    ("R", 0, 2, True), ("C", 2, 2, False)]

            with (
                tc.tile_pool(name="bcp", bufs=6) as bcp,
                tc.tile_pool(name="bcs", bufs=2) as bcs,
                tc.tile_pool(name="rp", bufs=10) as rp,
                tc.tile_pool(name="psum_g", bufs=8, space="PSUM") as psg,
                tc.tile_pool(name="gw", bufs=1) as gw,
            ):
                for gp in range(2):
                    glo, ghi = gp * 2048, (gp + 1) * 2048
                    ps = [[psg.tile([128, 512], F32, tag="psb",
                                    name=f"ps{gp}_{b}_{o}")
                           for o in range(2)] for b in range(4)]
                    for k in range(KK):
                        kops = _k_ops(k)
                        bct = []
                        for (pair, xs, r0, c0, rl, rh) in kops:
                            if pair:
                                t = bcp.tile([128, 2048], BF16, tag="bcp128",
                                             name="bcpair")
                            else:
                                t = bcs.tile([64, 2048], BF16, tag="bcs64",
                                             name="bcsolo")
                            bc_dma(t[0:64, :],
                                   psi_dram[rl:rl + 1, glo:ghi].to_broadcast(
                                       [64, 2048]))
                            if pair:
                                bc_dma(t[64:128, :],
                                       psi_dram[rh:rh + 1, glo:ghi]
                                       .to_broadcast([64, 2048]))
                            bct.append(t)
                        for gh in range(2):
                            G = gp * 2 + gh
                            rts = []
                            for oi, (pair, xs, r0, c0, rl, rh) in \
                                    enumerate(kops):
                                sel = xs if c0 % 2 == 0 else xs + "b"
                                cc = c0 if c0 % 2 == 0 else c0 - 1
                                np_ = 128 if pair else 64
                                rt = rp.tile([128, 1024], BF16, tag="rt")
                                if not pair:
                                    nc.vector.memset(rt[64:128, :], 0.0)
                                nc.vector.tensor_mul(
                                    rt[0:np_, :].rearrange(
                                        "p (r c) -> p r c", c=W),
                                    bct[oi][0:np_, gh * 1024:(gh + 1) * 1024]
                                    .rearrange("p (r c) -> p r c", c=W),
                                    xg[sel][0:np_,
                                            G * 16 + r0:G * 16 + r0 + 16,
                                            cc:cc + W])
                                rts.append(rt)
                            for oh in range(2):
                                st = wdcn[:, k, oh * 128:(oh + 1) * 128]
                                for oi, rt in enumerate(rts):
                                    for cb in range(2):
                                        nc.tensor.matmul(
                                            ps[gh * 2 + cb][oh][:], st,
                                            rt[:, cb * 512:(cb + 1) * 512],
                                            start=(k == 0 and oi == 0),
                                            stop=False)

                    # h-conv for the 4 blocks of this group-pair
                    for oh in range(2):
                        for j, (hs, ky, kx, pair) in enumerate(hops):
                            hv = hgC if hs == "C" else hgR
                            np_ = 128 if pair else 64
                            st = whp[0:np_, j, oh * 128:(oh + 1) * 128]
                            for bl in range(4):
                                blk = gp * 4 + bl
                                nc.tensor.matmul(
                                    ps[bl][oh][:], st,
                                    hv[0:np_, blk * 8 + ky:blk * 8 + ky + 8,
                                       kx:kx + W],
                                    start=False, stop=(j == len(hops) - 1))

                    # gates per block
                    for bl in range(4):
                        blk = gp * 4 + bl
                        lo, hi = blk * 512, (blk + 1) * 512
                        ps0, ps1 = ps[bl][0], ps[bl][1]
                        tif = gw.tile([128, 512], F32, tag="tif")
                        uif = gw.tile([128, 512], F32, tag="uif")
                        uif2 = gw.tile([128, 512], F32, tag="uif2")
                        ift = gw.tile([128, 512], F32, tag="ift")
                        cgc = gw.tile([128, 512], F32, tag="cgc")
                        tc_ = gw.tile([64, 512], F32, tag="tc_")
                        prod = gw.tile([64, 512], F32, tag="prod")
                        pf = gw.tile([64, 512], F32, tag="pf")
                        cnx = gw.tile([64, 512], F32, tag="cnx")
                        to_ = gw.tile([64, 512], F32, tag="to")
                        to2 = gw.tile([64, 512], F32, tag="to2")
                        uo = gw.tile([64, 512], F32, tag="uo")
                        ot = gw.tile([64, 512], F32, tag="ot")
                        rc = gw.tile([64, 512], F32, tag="rc")
                        hnx = gw.tile([64, 512], F32, tag="hnx")

                        nc.vector.tensor_mul(tif[:], mulc_if[:, lo:hi],
                                             c2[:, lo:hi])
                        nc.vector.scalar_tensor_tensor(
                            uif[:], ps0[:], 1.0, tif[:], OP.mult, OP.add)
                        nc.vector.tensor_add(uif2[:], uif[:], PALL0[:, lo:hi])
                        nc.scalar.activation(ift[:], uif2[:], AF.Sigmoid,
                                             bias=bdcn0)
                        nc.vector.tensor_add(tc_[:], ps1[0:64, :],
                                             PCOc[0:64, lo:hi])
                        nc.scalar.activation(cgc[0:64, :], tc_[:], AF.Relu,
                                             bias=consts[0:64, 2:3])
                        nc.scalar.activation(cgc[64:128, :],
                                             c2[64:128, lo:hi], AF.Copy)
                        nc.vector.tensor_mul(prod[:], ift[0:64, :],
                                             cgc[0:64, :])
                        nc.vector.tensor_mul(pf[:], ift[64:128, :],
                                             cgc[64:128, :])
                        nc.vector.tensor_add(cnx[:], prod[:], pf[:])
                        nc.vector.tensor_mul(to_[:], mulc_o[:, lo:hi],
                                             cnx[:])
                        nc.vector.tensor_add(to2[:], to_[:],
                                             PCOo[0:64, lo:hi])
                        nc.vector.scalar_tensor_tensor(
                            uo[:], ps1[64:128, :], 1.0, to2[:],
                            OP.mult, OP.add)
                        nc.scalar.activation(ot[:], uo[:], AF.Sigmoid,
                                             bias=consts[64:128, 2:3])
                        nc.scalar.activation(rc[:], cnx[:], AF.Relu)
                        nc.vector.tensor_mul(hnx[:], ot[:], rc[:])
                        nc.sync.dma_start(c_out[:, lo:hi], cnx[:])
                        nc.sync.dma_start(h_out[:, lo:hi], hnx[:])

    nc.compile()
    return nc


# ---------------- host side ----------------

def _host_om(x, w_off, b_off):
    x = np.asarray(x, np.float32)
    w = np.asarray(w_off, np.float32)
    bb = np.asarray(b_off, np.float32)
    xp = np.pad(x, ((0, 0), (0, 0), (1, 1), (1, 1)))
    om = np.zeros((B, 3 * KK, H, W), np.float32)
    for ky in range(3):
        for kx in range(3):
            om += np.einsum("oc,bchw->bohw", w[:, :, ky, kx],
                            xp[:, :, ky:ky + H, kx:kx + W], optimize=True)
    return om + bb[None, :, None, None]


def _tent(d, j):
    a1 = max(d, 0.0)
    a2 = max(d - 1.0, 0.0)
    b1 = max(-d, 0.0)
    b2 = max(-d - 1.0, 0.0)
    return (b2, b1 - 2 * b2, max(1.0 - a1 - b1, 0.0), a1 - 2 * a2, a2)[j]


def compute_corr(x, w_off, b_off):
    """Union correction structure + per-core coefficient values."""
    om = _host_om(x, w_off, b_off)
    dy, dx = om[:, :KK], om[:, KK:2 * KK]
    mask = 1.0 / (1.0 + np.exp(-om[:, 2 * KK:]))
    PR = 2e-3

    uniq = {}   # (g, p) -> slot list [(k, u, v, nbr)]
    vals = {}   # (b, k, u, v, p) -> value
    viol = (np.abs(dy) > 1.0) | (np.abs(dx) > 1.0)
    bidx, kidx, ridx, widx = np.nonzero(viol)
    for b, k, r, c in zip(bidx, kidx, ridx, widx):
        kh, kw = k // 3 - 1, k % 3 - 1
        dyv = float(dy[b, k, r, c])
        dxv = float(dx[b, k, r, c])
        mv = float(mask[b, k, r, c])
        assert abs(dyv) < 2.0 and abs(dxv) < 2.0, "offset >= 2 unsupported"
        p = int(r) * W + int(c)
        for ju in range(5):
            for jv in range(5):
                u, v = ju - 2, jv - 2
                if abs(u) != 2 and abs(v) != 2:
                    continue
                val = _tent(dyv, ju) * _tent(dxv, jv) * mv
                if abs(val) < PR:
                    continue
                nr, ncol = int(r) + kh + u, int(c) + kw + v
                if not (0 <= nr < H and 0 <= ncol < W):
                    continue
                slots = uniq.setdefault((k // 2, p), [])
                sk = (int(k), u, v, nr * W + ncol)
                if sk not in slots:
                    slots.append(sk)
                vals[(int(b), int(k), u, v, p)] = val

    normal, ovl = [], []   # (g, p, slots<=4)
    seen = set()
    for (g, p), slots in sorted(uniq.items()):
        chunks = [slots[i:i + 4] for i in range(0, len(slots), 4)]
        for ci, chunk in enumerate(chunks):
            if ci == 0 and p not in seen:
                normal.append((g, p, chunk))
                seen.add(p)
            else:
                ovl.append((g, p, chunk))

    bygroup = [[] for _ in range(5)]
    for (g, p, slots) in normal:
        bygroup[g].append((p, slots, False))
    for (g, p, slots) in ovl:
        bygroup[g].append((p, slots, True))

    granges = []
    gidx = np.zeros(NC4, np.int64)
    sidx = np.full(NUP, HW, np.int64)   # HW -> skipped by bounds check
    ov = []
    colinfo = []
    j = 0
    for g in range(5):
        s = j
        for (p, slots, is_ov) in bygroup[g]:
            assert j < NUP, "too many correction columns"
            for si, (k, u, v, nbr) in enumerate(slots):
                gidx[4 * j + si] = nbr
            if is_ov:
                ov.append((j, p))
            else:
                sidx[j] = p
            colinfo.append((j, p, slots))
            j += 1
        granges.append([s, j])
    granges[4][1] = NUP  # pad columns keep PSUM fully written
    assert len(ov) <= 8, f"too many overflow columns: {len(ov)}"

    corr4 = np.zeros((B, 2, NC4), np.float32)
    for (jj, p, slots) in colinfo:
        for si, (k, u, v, nbr) in enumerate(slots):
            half = k % 2
            for b in range(B):
                val = vals.get((b, k, u, v, p))
                if val:
                    corr4[b, half, 4 * jj + si] = val

    gidx128 = np.zeros((128, NC4 // 128), np.int32)
    for i in range(NC4):
        gidx128[i % 128, i // 128] = gidx[i]
    sidx128 = np.zeros((128, NUP // 128), np.int32)
    for i in range(NUP):
        sidx128[i % 128, i // 128] = sidx[i]
    return (tuple(tuple(r) for r in granges), tuple(ov), gidx128, sidx128,
            corr4.astype(ml_dtypes.bfloat16))


def make_in_maps(x, h, c, w_off, b_off, w_dcn, b_dcn, w_h, mul_c,
                 gidx128, sidx128, corr4):
    x = np.ascontiguousarray(x, np.float32)
    h = np.ascontiguousarray(h, np.float32)
    c = np.ascontiguousarray(c, np.float32)
    mul_c = np.asarray(mul_c, np.float32)

    mulc_if = np.ascontiguousarray(
        mul_c[0, 0:128].reshape(128, HW)).astype(ml_dtypes.bfloat16)
    mulc_o = np.ascontiguousarray(
        mul_c[0, 128:192].reshape(64, HW)).astype(ml_dtypes.bfloat16)
    woff = np.ascontiguousarray(
        np.asarray(w_off, np.float32).reshape(27, 64, KK)
        .transpose(1, 2, 0)).astype(ml_dtypes.bfloat16)
    boff = np.asarray(b_off, np.float32).reshape(27, 1)
    wd = np.asarray(w_dcn, np.float32).reshape(256, 64, KK)
    wdcnp = np.zeros((128, KK, 256), np.float32)
    for k in range(KK):
        wk = wd[:, :, k].T
        wdcnp[0:64, k] = wk
        wdcnp[64:128, k] = wk
    wu = np.zeros((128, 5, 256), np.float32)
    for g in range(5):
        wu[0:64, g] = wd[:, :, 2 * g].T
        if 2 * g + 1 < KK:
            wu[64:128, g] = wd[:, :, 2 * g + 1].T
    bdcn = np.ascontiguousarray(
        np.asarray(b_dcn, np.float32).reshape(2, 128).T)
    wh = np.asarray(w_h, np.float32).reshape(256, 64, KK)
    whp = np.zeros((128, 5, 256), np.float32)
    for j, (ta, tb) in enumerate(((0, 1), (3, 4), (6, 7), (2, 5))):
        whp[0:64, j] = wh[:, :, ta].T
        whp[64:128, j] = wh[:, :, tb].T
    whp[0:64, 4] = wh[:, :, 8].T

    shared = dict(mulc_if=mulc_if, mulc_o=mulc_o, woff=woff, boff=boff,
                  wdcnp=wdcnp.astype(ml_dtypes.bfloat16), wu=wu.astype(ml_dtypes.bfloat16),
                  bdcn=bdcn, whp=whp.astype(ml_dtypes.bfloat16),
                  gidx=gidx128, sidx=sidx128)
    def padx(xb, a, bc):
        return np.pad(xb, ((0, 0), (a, 6 - a), (bc, 6 - bc))).reshape(
            C, XP * XP)

    def padh(hb, a, bc):
        return np.pad(hb, ((0, 0), (a, 2 - a), (bc, 2 - bc))).reshape(
            C, HP * HP)

    in_maps = []
    for b in range(B):
        m = dict(shared)
        xb = x[b].astype(ml_dtypes.bfloat16)
        hb = h[b].astype(ml_dtypes.bfloat16)
        m["xt16"] = np.ascontiguousarray(xb.reshape(C, HW).T)
        m["xpc"] = np.vstack([padx(xb, 3, 3), padx(xb, 3, 2)])
        m["xpcb"] = np.vstack([padx(xb, 3, 2), padx(xb, 3, 1)])
        m["xpr"] = np.vstack([padx(xb, 3, 3), padx(xb, 2, 3)])
        m["xprb"] = np.vstack([padx(xb, 3, 2), padx(xb, 2, 2)])
        m["hpc"] = np.vstack([padh(hb, 1, 1), padh(hb, 1, 0)])
        m["hpr"] = np.vstack([padh(hb, 1, 1), padh(hb, 0, 1)])
        m["cf"] = c[b].reshape(C, HW)
        m["corr4"] = corr4[b].astype(ml_dtypes.bfloat16)
        in_maps.append(m)
    return in_maps


def get_nc(granges, ov):
    key = (granges, ov)
    if key not in _COMPILED:
        _COMPILED[key] = _build(granges, ov)
    return _COMPILED[key]


def kernel(x, h, c, w_off, b_off, w_dcn, b_dcn, w_h, mul_c):
    granges, ov, gidx128, sidx128, corr4 = compute_corr(x, w_off, b_off)
    nc = get_nc(granges, ov)
    in_maps = make_in_maps(x, h, c, w_off, b_off, w_dcn, b_dcn, w_h, mul_c,
                           gidx128, sidx128, corr4)
    res = run_bass_kernel_spmd(nc, in_maps, core_ids=list(range(B)))
    h_next = np.stack([res.results[b]["h_out"].reshape(C, H, W)
                       for b in range(B)])
    c_next = np.stack([res.results[b]["c_out"].reshape(C, H, W)
                       for b in range(B)])
    return h_next.astype(np.float32), c_next.astype(np.float32)
